# revision 1
# baseline (speedup 1.0000x reference)
"""Local (sparse) attention layer on 8 Trainium2 NeuronCores.

Sharding: core c handles batch b = c//2, query half c%2 (1024 queries),
full context of its batch (data parallel on the small Dense weights).

v4 pipeline (per core), all heavy data in bf16:
  Host prep: x^T, ctx^T, weights pre-cast to bf16 (Wq/bq pre-scaled by
  1/sqrt(hd)); neighbor indices pre-wrapped into the dma_gather int16
  channel layout.
  A. PE projections straight from the host-transposed activations:
     q (n-major, SBUF), packed [k|v] bf16 rows -> HBM.
  B. Per 128-query tile, per 16-neighbor half-gather j (software
     pipelined: score phase of j overlaps AV phase of j-1):
       score: dma_gather 2x1024 kv rows; DVE dot-products via in-place
         multiply + bf16 pairwise tree over head_dim (TensorReduce runs
         at 1 elem/cycle, the tree runs at 2x); Act exponentiates with
         broadcast over head_dim into an expanded weight tile.
       AV: DVE weights V rows in-place and tree-reduces over neighbors
         (DVE for the big rounds, Pool for the small ones), f32
         accumulation across the two halves (exact flash accumulation:
         scores are O(1), no max shift needed).
     Normalize, PE out-projection, DMA out.
"""

import numpy as np

HEADS = 8
HD = 64
DIM = 512
DIN = 256
B, N, M, K = 4, 2048, 2048, 32
N_LOC = 1024  # queries per core
NT = N_LOC // 128  # query tiles per core
KH = 16  # neighbors per half-gather
NH = K // KH  # half-gathers per tile (2)
NJ = NT * NH  # pipelined half-gather stages

_CACHE = {}


def _build():
    import concourse.bass as bass
    import concourse.bacc as bacc
    import concourse.mybir as mybir
    from concourse.tile import TileContext
    from concourse.masks import make_identity

    f32 = mybir.dt.float32
    bf16 = mybir.dt.bfloat16
    i16 = mybir.dt.int16

    nc = bacc.Bacc("TRN2")
    xT_h = nc.dram_tensor("xT", [DIN, N_LOC], bf16, kind="ExternalInput")
    cT_h = nc.dram_tensor("cT", [DIN, M], bf16, kind="ExternalInput")
    idx_h = nc.dram_tensor("idx", [128, NJ * 128], i16, kind="ExternalInput")
    wq_h = nc.dram_tensor("wq", [DIN, DIM], bf16, kind="ExternalInput")
    wk_h = nc.dram_tensor("wk", [DIN, DIM], bf16, kind="ExternalInput")
    wv_h = nc.dram_tensor("wv", [DIN, DIM], bf16, kind="ExternalInput")
    wo_h = nc.dram_tensor("wo", [DIM, DIN], bf16, kind="ExternalInput")
    bq_h = nc.dram_tensor("bq", [128, DIM], f32, kind="ExternalInput")
    bo_h = nc.dram_tensor("bo", [128, DIN], f32, kind="ExternalInput")
    out_h = nc.dram_tensor("out", [N_LOC, DIN], f32, kind="ExternalOutput")
    kv_h = nc.dram_tensor("kv_scratch", [M, 2 * DIM], bf16, kind="Internal")

    with TileContext(nc) as tc:
        with tc.tile_pool(name="const", bufs=1) as cpool:
            ident = cpool.tile([128, 128], bf16)
            make_identity(nc, ident[:])
            wq_sb = [cpool.tile([128, DIM], bf16, tag=f"wq{c}", name=f"wq{c}") for c in range(2)]
            wk_sb = [cpool.tile([128, DIM], bf16, tag=f"wk{c}", name=f"wk{c}") for c in range(2)]
            wv_sb = [cpool.tile([128, DIM], bf16, tag=f"wv{c}", name=f"wv{c}") for c in range(2)]
            wo_sb = [cpool.tile([128, DIN], bf16, tag=f"wo{c}", name=f"wo{c}") for c in range(4)]
            bq_sb = cpool.tile([128, DIM], f32)
            bo_sb = cpool.tile([128, DIN], f32)
            idx_sb = cpool.tile([128, NJ * 128], i16)
            for c in range(2):
                nc.sync.dma_start(out=wk_sb[c][:], in_=wk_h[c * 128:(c + 1) * 128, :])
                nc.sync.dma_start(out=wv_sb[c][:], in_=wv_h[c * 128:(c + 1) * 128, :])
            for c in range(2):
                nc.sync.dma_start(out=wq_sb[c][:], in_=wq_h[c * 128:(c + 1) * 128, :])
            nc.sync.dma_start(out=idx_sb[:], in_=idx_h[:])
            for c in range(4):
                nc.sync.dma_start(out=wo_sb[c][:], in_=wo_h[c * 128:(c + 1) * 128, :])
            nc.sync.dma_start(out=bq_sb[:], in_=bq_h[:])
            nc.sync.dma_start(out=bo_sb[:], in_=bo_h[:])

            with tc.tile_pool(name="qpool", bufs=1) as qpool:
                q_sb = [qpool.tile([128, DIM], bf16, tag=f"q{t}", name=f"q{t}") for t in range(NT)]

                # ---- phase A: projections ----
                with (
                    tc.tile_pool(name="inp", bufs=1) as ipool,
                    tc.tile_pool(name="stage", bufs=4) as stpool,
                    tc.tile_pool(name="psA", bufs=2, space="PSUM") as psA,
                ):
                    xT_sb = [ipool.tile([128, N_LOC], bf16, tag=f"xT{c}", name=f"xT{c}") for c in range(2)]
                    cT_sb = [ipool.tile([128, M], bf16, tag=f"cT{c}", name=f"cT{c}") for c in range(2)]
                    for c in range(2):
                        nc.sync.dma_start(out=xT_sb[c][:], in_=xT_h[c * 128:(c + 1) * 128, :])
                        nc.sync.dma_start(out=cT_sb[c][:], in_=cT_h[c * 128:(c + 1) * 128, :])
                    for mt in range(M // 128):
                        psk = psA.tile([128, DIM], f32, tag="mmk")
                        psv = psA.tile([128, DIM], f32, tag="mmv")
                        for c in range(2):
                            nc.tensor.matmul(
                                out=psk[:], lhsT=cT_sb[c][:, mt * 128:(mt + 1) * 128],
                                rhs=wk_sb[c][:], start=(c == 0), stop=(c == 1))
                        for c in range(2):
                            nc.tensor.matmul(
                                out=psv[:], lhsT=cT_sb[c][:, mt * 128:(mt + 1) * 128],
                                rhs=wv_sb[c][:], start=(c == 0), stop=(c == 1))
                        kvt = stpool.tile([128, 2 * DIM], bf16, tag="kvt")
                        if mt % 2 == 0:
                            nc.scalar.activation(
                                out=kvt[:, :DIM], in_=psk[:],
                                func=mybir.ActivationFunctionType.Copy)
                            nc.vector.tensor_copy(out=kvt[:, DIM:], in_=psv[:])
                        else:
                            nc.vector.tensor_copy(out=kvt[:, :DIM], in_=psk[:])
                            nc.scalar.activation(
                                out=kvt[:, DIM:], in_=psv[:],
                                func=mybir.ActivationFunctionType.Copy)
                        nc.sync.dma_start(
                            out=kv_h[mt * 128:(mt + 1) * 128, :], in_=kvt[:])
                    for t in range(NT):
                        psq = psA.tile([128, DIM], f32, tag="mmq")
                        for c in range(2):
                            nc.tensor.matmul(
                                out=psq[:], lhsT=xT_sb[c][:, t * 128:(t + 1) * 128],
                                rhs=wq_sb[c][:], start=(c == 0), stop=(c == 1))
                        nc.vector.tensor_tensor(
                            out=q_sb[t][:], in0=psq[:], in1=bq_sb[:],
                            op=mybir.AluOpType.add)

                # ---- phase B: software-pipelined gather + attention ----
                with (
                    tc.tile_pool(name="gat", bufs=4) as gpool,
                    tc.tile_pool(name="sco", bufs=3) as spool,
                    tc.tile_pool(name="eexp", bufs=2) as epool,
                    tc.tile_pool(name="red", bufs=3) as rpool,
                    tc.tile_pool(name="acc", bufs=3) as apool,
                    tc.tile_pool(name="psT", bufs=4, space="PSUM") as psT,
                    tc.tile_pool(name="psO", bufs=4, space="PSUM") as psO,
                ):
                    kvgs, eexs, sts = {}, {}, {}
                    avs, dens = {}, {}

                    def gather_phase(j):
                        kvg = gpool.tile([128, KH, 2 * DIM], bf16, tag="kvg", name="kvg")
                        kvgs[j] = kvg
                        for g in range(2):
                            col0 = j * 128 + g * 64
                            nc.gpsimd.dma_gather(
                                out_ap=kvg[:, g * (KH // 2):(g + 1) * (KH // 2), :],
                                in_ap=kv_h[:],
                                idxs_ap=idx_sb[:, col0:col0 + 64],
                                num_idxs=KH * 64,
                                num_idxs_reg=KH * 64,
                                elem_size=2 * DIM,
                            )

                    def score_phase(j):
                        t = j // NH
                        kvg = kvgs[j]
                        kg = kvg[:, :, :DIM].rearrange(
                            "p k (h d) -> p k h d", h=HEADS)
                        # scores in-place into the gathered K half
                        nc.vector.tensor_tensor(
                            out=kg, in0=kg,
                            in1=q_sb[t][:].rearrange(
                                "p (o h d) -> p o h d", o=1, h=HEADS
                            ).to_broadcast([128, KH, HEADS, HD]),
                            op=mybir.AluOpType.mult)
                        # bf16 pairwise tree over head_dim (2x DVE rate),
                        # then a small f32 reduce over the last 4
                        with nc.allow_low_precision(reason="bf16 dot tree, f32 finish"):
                            for eng, w in ((nc.vector, 32), (nc.vector, 16), (nc.vector, 8), (nc.vector, 4)):
                                eng.tensor_tensor(
                                    out=kg[:, :, :, :w], in0=kg[:, :, :, :w],
                                    in1=kg[:, :, :, w:2 * w],
                                    op=mybir.AluOpType.add)

                    def score_back(j):
                        kvg = kvgs[j]
                        kg = kvg[:, :, :DIM].rearrange(
                            "p k (h d) -> p k h d", h=HEADS)
                        s = spool.tile([128, KH, HEADS], f32, tag="s", name="s")
                        sts[j] = s
                        nc.vector.tensor_reduce(
                            out=s[:], in_=kg[:, :, :, :4],
                            axis=mybir.AxisListType.X,
                            op=mybir.AluOpType.add)
                        # exp(s) broadcast-expanded over head_dim (Act)
                        eex = epool.tile([128, KH, HEADS, HD], bf16, tag="eex", name="eex")
                        eexs[j] = eex
                        nc.scalar.activation(
                            out=eex[:],
                            in_=s[:].rearrange(
                                "p k (h o) -> p k h o", o=1
                            ).to_broadcast([128, KH, HEADS, HD]),
                            func=mybir.ActivationFunctionType.Exp)

                    def av_phase(j):
                        t, h2 = j // NH, j % NH
                        kvg, eex = kvgs.pop(j), eexs.pop(j)
                        if h2 == 0:
                            avs[t] = apool.tile([128, DIM], f32, tag="av", name="av")
                            dens[t] = apool.tile([128, HEADS], f32, tag="den", name="den")
                        av, den = avs[t], dens[t]
                        # weight V rows in-place (DVE, fully packed bf16)
                        nc.vector.tensor_tensor(
                            out=kvg[:, :, DIM:],
                            in0=kvg[:, :, DIM:],
                            in1=eex[:].rearrange("p k h d -> p k (h d)"),
                            op=mybir.AluOpType.mult)
                        # pairwise tree-reduce over the 16 neighbors
                        with nc.allow_low_precision(reason="bf16 flash accum, f32 final"):
                            nc.vector.tensor_tensor(
                                out=kvg[:, :8, DIM:], in0=kvg[:, :8, DIM:],
                                in1=kvg[:, 8:, DIM:], op=mybir.AluOpType.add)
                            nc.gpsimd.tensor_tensor(
                                out=kvg[:, :4, DIM:], in0=kvg[:, :4, DIM:],
                                in1=kvg[:, 4:8, DIM:], op=mybir.AluOpType.add)
                            nc.gpsimd.tensor_tensor(
                                out=kvg[:, :2, DIM:], in0=kvg[:, :2, DIM:],
                                in1=kvg[:, 2:4, DIM:], op=mybir.AluOpType.add)
                        dpart = spool.tile([128, HEADS], f32, tag="dpart", name="dpart")
                        nc.vector.tensor_reduce(
                            out=(den[:] if h2 == 0 else dpart[:]),
                            in_=eex[:, :, :, 0].rearrange("p k h -> p h k"),
                            axis=mybir.AxisListType.X,
                            op=mybir.AluOpType.add)
                        if h2 == 0:
                            nc.gpsimd.tensor_tensor(
                                out=av[:], in0=kvg[:, 0, DIM:], in1=kvg[:, 1, DIM:],
                                op=mybir.AluOpType.add)
                        else:
                            rsum = rpool.tile([128, DIM], f32, tag="rsum", name="rsum")
                            nc.gpsimd.tensor_tensor(
                                out=rsum[:], in0=kvg[:, 0, DIM:], in1=kvg[:, 1, DIM:],
                                op=mybir.AluOpType.add)
                            nc.gpsimd.tensor_tensor(
                                out=av[:], in0=av[:], in1=rsum[:],
                                op=mybir.AluOpType.add)
                            nc.vector.tensor_tensor(
                                out=den[:], in0=den[:], in1=dpart[:],
                                op=mybir.AluOpType.add)

                    aos = {}

                    def tail1(t):
                        av, den = avs.pop(t), dens.pop(t)
                        rden = apool.tile([128, HEADS], f32, tag="rden", name="rden")
                        nc.vector.reciprocal(out=rden[:], in_=den[:])
                        ao = apool.tile([128, DIM], bf16, tag="ao", name="ao")
                        aos[t] = ao
                        nc.gpsimd.tensor_tensor(
                            out=ao[:].rearrange("p (h d) -> p h d", h=HEADS),
                            in0=av[:].rearrange("p (h d) -> p h d", h=HEADS),
                            in1=rden[:].rearrange(
                                "p (h o) -> p h o", o=1).to_broadcast([128, HEADS, HD]),
                            op=mybir.AluOpType.mult)

                    def tail2(t):
                        ao = aos.pop(t)
                        pst = psT.tile([128, DIM], bf16, tag="tp", name="pst")
                        for c in range(4):
                            nc.tensor.transpose(
                                out=pst[:, c * 128:(c + 1) * 128],
                                in_=ao[:, c * 128:(c + 1) * 128],
                                identity=ident[:])
                        aT = apool.tile([128, DIM], bf16, tag="aT", name="aT")
                        nc.scalar.activation(
                            out=aT[:], in_=pst[:],
                            func=mybir.ActivationFunctionType.Copy)
                        pso = psO.tile([128, DIN], f32, tag="mo", name="pso")
                        for c in range(4):
                            nc.tensor.matmul(
                                out=pso[:], lhsT=aT[:, c * 128:(c + 1) * 128],
                                rhs=wo_sb[c][:], start=(c == 0), stop=(c == 3))
                        ot = apool.tile([128, DIN], f32, tag="ot", name="ot")
                        nc.vector.tensor_tensor(
                            out=ot[:], in0=pso[:], in1=bo_sb[:],
                            op=mybir.AluOpType.add)
                        nc.sync.dma_start(
                            out=out_h[t * 128:(t + 1) * 128, :], in_=ot[:])

                    gather_phase(0)
                    gather_phase(1)
                    for j in range(NJ + 2):
                        if j + 2 < NJ:
                            gather_phase(j + 2)
                        if j < NJ:
                            score_phase(j)
                        if 1 <= j <= NJ:
                            av_phase(j - 1)
                        if j < NJ:
                            score_back(j)
                        if 1 <= j <= NJ and (j - 1) % NH == NH - 1:
                            tail1((j - 1) // NH)
                        if 2 <= j and (j - 2) % NH == NH - 1 and j - 2 >= 0:
                            tail2((j - 2) // NH)
    nc.compile()
    return nc


def _get_nc():
    if "nc" not in _CACHE:
        _CACHE["nc"] = _build()
    return _CACHE["nc"]


def kernel(**inputs) -> np.ndarray:
    from concourse.bass_utils import run_bass_kernel_spmd
    from ml_dtypes import bfloat16

    x = np.asarray(inputs["x"], dtype=np.float32)
    ctx = np.asarray(inputs["context"], dtype=np.float32)
    idx = np.asarray(inputs["index_pairs"]).astype(np.int64)
    scale = 1.0 / np.sqrt(HD)
    wq = (np.asarray(inputs["Wq"], dtype=np.float32) * scale).astype(bfloat16)
    bq = np.tile((np.asarray(inputs["bq"], dtype=np.float32) * scale).reshape(1, DIM),
                 (128, 1)).astype(np.float32)
    wk = np.asarray(inputs["Wk"], dtype=np.float32).astype(bfloat16)
    wv = np.asarray(inputs["Wv"], dtype=np.float32).astype(bfloat16)
    wo = np.asarray(inputs["Wout"], dtype=np.float32).astype(bfloat16)
    bo = np.tile(np.asarray(inputs["bout"], dtype=np.float32).reshape(1, DIN),
                 (128, 1)).astype(np.float32)

    nc = _get_nc()
    in_maps = []
    for c in range(8):
        b, half = c // 2, c % 2
        xT_c = np.ascontiguousarray(
            x[b, half * N_LOC:(half + 1) * N_LOC, :].T).astype(bfloat16)
        cT_c = np.ascontiguousarray(ctx[b].T).astype(bfloat16)
        idx_c = idx[b, half * N_LOC:(half + 1) * N_LOC, :].astype(np.int16)  # [1024, 32]
        # dma_gather channel layout: per (tile, half, quarter) block of 64
        # cols, item i = kk*128 + q lives at [i % 16, block*64 + i // 16],
        # replicated to all 8 GPSIMD-core partition groups.
        blocks = []
        for t in range(NT):
            for h2 in range(NH):
                for g in range(2):
                    k0 = h2 * KH + g * (KH // 2)
                    sub = idx_c[t * 128:(t + 1) * 128, k0:k0 + KH // 2]  # [128 q, 8 kk]
                    items = sub.T.reshape(-1)  # items[kk*128 + q]
                    blocks.append(items.reshape(64, 16).T)  # [16, 64]
        idx_w = np.tile(np.concatenate(blocks, axis=1), (8, 1))
        in_maps.append({
            "xT": xT_c, "cT": cT_c, "idx": idx_w,
            "wq": wq, "wk": wk, "wv": wv, "wo": wo, "bq": bq, "bo": bo,
        })
    res = run_bass_kernel_spmd(nc, in_maps, core_ids=list(range(8)))
    out = np.empty((B, N, DIN), dtype=np.float32)
    for c in range(8):
        b, half = c // 2, c % 2
        out[b, half * N_LOC:(half + 1) * N_LOC, :] = res.results[c]["out"]
    return out



# revision 14
# speedup vs baseline: 1.7270x; 1.7270x over previous
"""Local (sparse) attention layer on 8 Trainium2 NeuronCores.

Sharding: core c handles batch b = c//2, query half c%2 (1024 queries),
full context of its batch (data parallel on the small Dense weights).

v5 "fully dense, zero-gather" pipeline (per core):
  The 32-neighbor sparse attention is recast as dense attention against
  the full 2048-token context, masked by a host-built neighbor-COUNT
  matrix (exactly preserving duplicate-index multiplicity):

      out_q = (sum_c cnt[c,q] * exp(s[c,q]) * V[c]) / (sum_c cnt[c,q] * exp(s[c,q]))

  This trades the 64MB/core DMA row-gather of the previous version
  (~186us of DMA at the modeled 360B/ns) for dense PE matmuls, which are
  nearly free on the tensor engine, plus one dense exp pass on Act.

  Per core:
    A. PE projections from host-transposed activations: qT [512,1024],
       kT [512,2048] (hd-major), V [2048,512] (c-major) in SBUF, bf16.
       Wq/bq pre-scaled by 1/sqrt(hd) on the host.
    B. Per (query tile t, context block cb) unit:
       - PE: S^T[c, (h,q)] = kT_h^T qT_h  (8 matmuls, f32 PSUM)
       - Act: A = exp(S^T) -> SBUF bf16 (the only Act work; bottleneck)
       - DVE/GPSIMD: A *= cnt[c,q] (broadcast over heads)
       - PE: av[q, (h,d)] += A_h^T V_h and den[q,h] += A_h^T ones,
         accumulated over the 16 context blocks in PSUM.
    C. Tail per tile: normalize by 1/den (DVE), PE transpose, PE
       out-projection, bias add (DVE), DMA out. Output is q-major so no
       host-side transpose is needed.
"""

import numpy as np

HEADS = 8
HD = 64
DIM = 512
DIN = 256
B, N, M, K = 4, 2048, 2048, 32
N_LOC = 1024  # queries per core
NT = N_LOC // 128  # query tiles per core
NCB = M // 128  # context blocks

_CACHE = {}


def _build():
    import concourse.bass as bass
    import concourse.bacc as bacc
    import concourse.mybir as mybir
    from concourse.tile import TileContext
    from concourse.masks import make_identity

    f32 = mybir.dt.float32
    bf16 = mybir.dt.bfloat16

    nc = bacc.Bacc("TRN2")
    xT_h = nc.dram_tensor("xT", [DIN, N_LOC], bf16, kind="ExternalInput")
    cT_h = nc.dram_tensor("cT", [DIN, M], bf16, kind="ExternalInput")
    cnt_h = nc.dram_tensor("cnt", [128, NT * NCB * 128], bf16, kind="ExternalInput")
    wq_h = nc.dram_tensor("wq", [DIN, DIM], bf16, kind="ExternalInput")
    wk_h = nc.dram_tensor("wk", [DIN, DIM], bf16, kind="ExternalInput")
    wv_h = nc.dram_tensor("wv", [DIN, DIM], bf16, kind="ExternalInput")
    wo_h = nc.dram_tensor("wo", [DIM, DIN], bf16, kind="ExternalInput")
    bq_h = nc.dram_tensor("bq", [64, 8], f32, kind="ExternalInput")
    bo_h = nc.dram_tensor("bo", [128, DIN], f32, kind="ExternalInput")
    out_h = nc.dram_tensor("out", [N_LOC, DIN], f32, kind="ExternalOutput")

    with TileContext(nc) as tc:
        with tc.tile_pool(name="const", bufs=1) as cpool:
            ident = cpool.tile([128, 128], bf16)
            make_identity(nc, ident[:])
            ones_sb = cpool.tile([128, 1], bf16)
            nc.vector.memset(ones_sb[:], 1.0)
            wq_sb = [cpool.tile([128, DIM], bf16, tag=f"wq{c}", name=f"wq{c}") for c in range(2)]
            wk_sb = [cpool.tile([128, DIM], bf16, tag=f"wk{c}", name=f"wk{c}") for c in range(2)]
            wv_sb = [cpool.tile([128, DIM], bf16, tag=f"wv{c}", name=f"wv{c}") for c in range(2)]
            wo_sb = [cpool.tile([128, DIN], bf16, tag=f"wo{c}", name=f"wo{c}") for c in range(4)]
            bqc_sb = cpool.tile([64, 8], f32)
            bo_sb = cpool.tile([128, DIN], f32)
            for c in range(2):
                nc.sync.dma_start(out=wq_sb[c][:], in_=wq_h[c * 128:(c + 1) * 128, :])
                nc.sync.dma_start(out=wk_sb[c][:], in_=wk_h[c * 128:(c + 1) * 128, :])
                nc.sync.dma_start(out=wv_sb[c][:], in_=wv_h[c * 128:(c + 1) * 128, :])
            for c in range(4):
                nc.sync.dma_start(out=wo_sb[c][:], in_=wo_h[c * 128:(c + 1) * 128, :])
            nc.sync.dma_start(out=bqc_sb[:], in_=bq_h[:])
            nc.sync.dma_start(out=bo_sb[:], in_=bo_h[:])

            with tc.tile_pool(name="perm", bufs=1) as ppool:
                # per-head [64, .] tiles: matmul operands must start at
                # partition 0 (offset-64 operands fail on device)
                qT_sb = [ppool.tile([64, N_LOC], bf16, tag=f"qT{h}", name=f"qT{h}") for h in range(8)]
                kT_sb = [ppool.tile([64, M], bf16, tag=f"kT{h}", name=f"kT{h}") for h in range(8)]
                v_sb = [ppool.tile([128, DIM], bf16, tag=f"v{cb}", name=f"v{cb}") for cb in range(NCB)]

                # ---- phase A: projections ----
                with (
                    tc.tile_pool(name="inp", bufs=1) as ipool,
                    tc.tile_pool(name="psA", bufs=3, space="PSUM") as psA,
                ):
                    xT_sb = [ipool.tile([128, N_LOC], bf16, tag=f"xT{c}", name=f"xT{c}") for c in range(2)]
                    cT_sb = [ipool.tile([128, M], bf16, tag=f"cT{c}", name=f"cT{c}") for c in range(2)]
                    for c in range(2):
                        nc.sync.dma_start(out=xT_sb[c][:], in_=xT_h[c * 128:(c + 1) * 128, :])
                        nc.sync.dma_start(out=cT_sb[c][:], in_=cT_h[c * 128:(c + 1) * 128, :])
                    # qT_h[d, q] = Wq_h^T x^T (+ bq, per-partition scalar)
                    for h in range(8):
                        for cc in range(2):
                            psq = psA.tile([64, 512], f32, tag="pAh")
                            for c in range(2):
                                nc.tensor.matmul(
                                    out=psq[:],
                                    lhsT=wq_sb[c][:, h * 64:(h + 1) * 64],
                                    rhs=xT_sb[c][:, cc * 512:(cc + 1) * 512],
                                    start=(c == 0), stop=(c == 1))
                            nc.vector.tensor_scalar(
                                out=qT_sb[h][:, cc * 512:(cc + 1) * 512],
                                in0=psq[:], scalar1=bqc_sb[:, h:h + 1], scalar2=None,
                                op0=mybir.AluOpType.add)
                    # kT_h[d, c] = Wk_h^T ctx^T
                    for h in range(8):
                        for cc in range(4):
                            psk = psA.tile([64, 512], f32, tag="pAh")
                            for c in range(2):
                                nc.tensor.matmul(
                                    out=psk[:],
                                    lhsT=wk_sb[c][:, h * 64:(h + 1) * 64],
                                    rhs=cT_sb[c][:, cc * 512:(cc + 1) * 512],
                                    start=(c == 0), stop=(c == 1))
                            nc.vector.tensor_copy(
                                out=kT_sb[h][:, cc * 512:(cc + 1) * 512], in_=psk[:])
                    # V[c, hd] = ctx Wv
                    for cb in range(NCB):
                        psv = psA.tile([128, 512], f32, tag="pA")
                        for c in range(2):
                            nc.tensor.matmul(
                                out=psv[:],
                                lhsT=cT_sb[c][:, cb * 128:(cb + 1) * 128],
                                rhs=wv_sb[c][:],
                                start=(c == 0), stop=(c == 1))
                        nc.vector.tensor_copy(out=v_sb[cb][:], in_=psv[:])

                # ---- phase B: dense attention ----
                with (
                    tc.tile_pool(name="cntp", bufs=2) as cntp,
                    tc.tile_pool(name="atp", bufs=10) as atp,
                    tc.tile_pool(name="tailp", bufs=2) as tailp,
                    tc.tile_pool(name="psS", bufs=2, space="PSUM") as psS,
                    tc.tile_pool(name="psav", bufs=1, space="PSUM") as psav,
                    tc.tile_pool(name="psden", bufs=1, space="PSUM") as psden,
                    tc.tile_pool(name="psT", bufs=1, space="PSUM") as psT,
                    tc.tile_pool(name="psO", bufs=1, space="PSUM") as psO,
                ):
                    uidx = 0
                    for t in range(NT):
                        cnt_sb = cntp.tile([128, NCB * 128], bf16, tag="cnt", name=f"cnt{t}")
                        nc.sync.dma_start(
                            out=cnt_sb[:], in_=cnt_h[:, t * 2048:(t + 1) * 2048])
                        av_ps = psav.tile([128, DIM], f32, tag="av", name=f"av{t}")
                        den_ps = psden.tile([128, 8], f32, tag="den", name=f"den{t}")
                        for cb in range(NCB):
                            # S^T[c, (h,q)] for this (t, cb)
                            s_ps = psS.tile([128, 1024], f32, tag="s", name=f"s{t}_{cb}")
                            for h in range(8):
                                nc.tensor.matmul(
                                    out=s_ps[:, h * 128:(h + 1) * 128],
                                    lhsT=kT_sb[h][:, cb * 128:(cb + 1) * 128],
                                    rhs=qT_sb[h][:, t * 128:(t + 1) * 128],
                                    start=True, stop=True)
                            # A = exp(S^T) (Act), then A *= cnt (DVE/GPSIMD)
                            slab = atp.tile([128, 8, 128], bf16, tag="at", name=f"at{t}_{cb}")
                            nc.scalar.activation(
                                out=slab[:].rearrange("p h q -> p (h q)"),
                                in_=s_ps[:],
                                func=mybir.ActivationFunctionType.Exp)
                            eng = nc.vector  # gpsimd offload disabled (device debug)
                            with nc.allow_low_precision(reason="bf16 attention weights"):
                                eng.tensor_tensor(
                                    out=slab[:], in0=slab[:],
                                    in1=cnt_sb[:, cb * 128:(cb + 1) * 128].rearrange(
                                        "p (o q) -> p o q", o=1
                                    ).to_broadcast([128, 8, 128]),
                                    op=mybir.AluOpType.mult)
                            uidx += 1
                            # av[q, (h,d)] += A_h^T V_h ; den[q, h] += A_h^T 1
                            # PSUM accumulation groups are per 2KB bank: open
                            # each bank's group on the very first matmul, close
                            # on the very last (intermediate order commutes).
                            for h in range(8):
                                nc.tensor.matmul(
                                    out=av_ps[:, h * 64:(h + 1) * 64],
                                    lhsT=slab[:, h, :],
                                    rhs=v_sb[cb][:, h * 64:(h + 1) * 64],
                                    start=(cb == 0 and h == 0),
                                    stop=(cb == NCB - 1 and h == 7))
                                nc.tensor.matmul(
                                    out=den_ps[:, h:h + 1],
                                    lhsT=slab[:, h, :],
                                    rhs=ones_sb[:],
                                    start=(cb == 0 and h == 0),
                                    stop=(cb == NCB - 1 and h == 7))
                        # ---- tail: normalize + out-projection ----
                        rden = tailp.tile([128, 8], f32, tag="rden", name=f"rden{t}")
                        nc.vector.reciprocal(out=rden[:], in_=den_ps[:])
                        ao = tailp.tile([128, DIM], bf16, tag="ao", name=f"ao{t}")
                        # inner-dim (d) stride-0 broadcast needs gpsimd (DVE
                        # lacks it on hw), and gpsimd can't read PSUM: stage
                        # av through SBUF first.
                        av_f = tailp.tile([128, DIM], f32, tag="avf", name=f"avf{t}")
                        nc.vector.tensor_copy(out=av_f[:], in_=av_ps[:])
                        with nc.allow_low_precision(reason="bf16 normalized attention out"):
                            nc.gpsimd.tensor_tensor(
                                out=ao[:].rearrange("p (h d) -> p h d", h=8),
                                in0=av_f[:].rearrange("p (h d) -> p h d", h=8),
                                in1=rden[:].rearrange("p (h o) -> p h o", o=1
                                                      ).to_broadcast([128, 8, 64]),
                                op=mybir.AluOpType.mult)
                        pst = psT.tile([128, DIM], bf16, tag="tp", name=f"pst{t}")
                        for c in range(4):
                            nc.tensor.transpose(
                                out=pst[:, c * 128:(c + 1) * 128],
                                in_=ao[:, c * 128:(c + 1) * 128],
                                identity=ident[:])
                        aT2 = tailp.tile([128, DIM], bf16, tag="aT2", name=f"aT2{t}")
                        nc.vector.tensor_copy(out=aT2[:], in_=pst[:])
                        op_ps = psO.tile([128, DIN], f32, tag="op", name=f"op{t}")
                        for c in range(4):
                            nc.tensor.matmul(
                                out=op_ps[:], lhsT=aT2[:, c * 128:(c + 1) * 128],
                                rhs=wo_sb[c][:], start=(c == 0), stop=(c == 3))
                        ot = tailp.tile([128, DIN], f32, tag="ot", name=f"ot{t}")
                        nc.vector.tensor_tensor(
                            out=ot[:], in0=op_ps[:], in1=bo_sb[:],
                            op=mybir.AluOpType.add)
                        nc.sync.dma_start(
                            out=out_h[t * 128:(t + 1) * 128, :], in_=ot[:])
    nc.compile()
    return nc


def _get_nc():
    if "nc" not in _CACHE:
        _CACHE["nc"] = _build()
    return _CACHE["nc"]


def kernel(**inputs) -> np.ndarray:
    from concourse.bass_utils import run_bass_kernel_spmd
    from ml_dtypes import bfloat16

    x = np.asarray(inputs["x"], dtype=np.float32)
    ctx = np.asarray(inputs["context"], dtype=np.float32)
    idx = np.asarray(inputs["index_pairs"]).astype(np.int64)
    scale = 1.0 / np.sqrt(HD)
    wq = (np.asarray(inputs["Wq"], dtype=np.float32) * scale).astype(bfloat16)
    bq = (np.asarray(inputs["bq"], dtype=np.float32) * scale).reshape(8, 64).T
    bq = np.ascontiguousarray(bq).astype(np.float32)  # [64, 8], col h = bq[h*64:(h+1)*64]
    wk = np.asarray(inputs["Wk"], dtype=np.float32).astype(bfloat16)
    wv = np.asarray(inputs["Wv"], dtype=np.float32).astype(bfloat16)
    wo = np.asarray(inputs["Wout"], dtype=np.float32).astype(bfloat16)
    bo = np.tile(np.asarray(inputs["bout"], dtype=np.float32).reshape(1, DIN),
                 (128, 1)).astype(np.float32)

    nc = _get_nc()
    in_maps = []
    qrep = np.repeat(np.arange(N_LOC), K)
    for c in range(8):
        b, half = c // 2, c % 2
        xT_c = np.ascontiguousarray(
            x[b, half * N_LOC:(half + 1) * N_LOC, :].T).astype(bfloat16)
        cT_c = np.ascontiguousarray(ctx[b].T).astype(bfloat16)
        idx_c = idx[b, half * N_LOC:(half + 1) * N_LOC, :]  # [1024, 32]
        # neighbor count matrix cnt[c, q], including duplicate multiplicity
        flat = idx_c.reshape(-1) * N_LOC + qrep
        cnt = np.bincount(flat, minlength=M * N_LOC).reshape(M, N_LOC)
        # layout [128 c-part, (t, cb, q)]
        cnt_w = np.ascontiguousarray(
            cnt.reshape(NCB, 128, NT, 128).transpose(1, 2, 0, 3).reshape(128, NT * NCB * 128)
        ).astype(bfloat16)
        in_maps.append({
            "xT": xT_c, "cT": cT_c, "cnt": cnt_w,
            "wq": wq, "wk": wk, "wv": wv, "wo": wo, "bq": bq, "bo": bo,
        })
    res = run_bass_kernel_spmd(nc, in_maps, core_ids=list(range(8)))
    out = np.empty((B, N, DIN), dtype=np.float32)
    for c in range(8):
        b, half = c // 2, c % 2
        out[b, half * N_LOC:(half + 1) * N_LOC, :] = res.results[c]["out"]
    return out


# revision 16
# speedup vs baseline: 1.8821x; 1.0898x over previous
"""Local (sparse) attention layer on 8 Trainium2 NeuronCores.

Sharding: core c handles batch b = c//2, query half c%2 (1024 queries),
full context of its batch (data parallel on the small Dense weights).

v6 "fully dense, zero-gather" pipeline (per core):
  The 32-neighbor sparse attention is recast as dense attention against
  the full 2048-token context, masked by a host-built neighbor-COUNT
  matrix (exactly preserving duplicate-index multiplicity):

      out_q = (sum_c cnt[c,q] exp(s[c,q]) V[c]) / (sum_c cnt[c,q] exp(s[c,q]))

  This trades the 64MB/core DMA row-gather of the v4 kernel (~186us at
  the modeled 360B/ns) for dense PE matmuls plus one dense exp pass on
  Act (the bottleneck: 128 x 1038ns back-to-back).

  Per core:
    A. PE projections from host-transposed activations; per-head
       qT_h [64,1024] / kT_h [64,2048] tiles (matmul operands must start
       at partition 0 on this device), V [2048,512] c-major. PSUM->SBUF
       copies split across DVE and Act (Act also folds the q bias via
       Identity+bias). Wq/bq pre-scaled by 1/sqrt(hd) on the host.
    B. Per (query tile t, context block cb) unit:
       - PE: S^T[c, (h,q)] = kT_h^T qT_h  (8 matmuls, f32 PSUM)
       - Act: A = exp(S^T) -> SBUF bf16
       - DVE: A *= cnt[c,q] (broadcast over heads)
       - PE: av[q,(h,d)] += A_h^T V_h ; den[q,h] += A_h^T ones
         (single PSUM accumulation group per bank across all 128 matmuls)
    C. Tail per tile, split so PSUM frees fast and PE never head-of-line
       blocks: tail1 (DVE: 1/den, av->SBUF) runs immediately; tail2
       (Pool normalize, PE transpose + out-projection, bias, DMA out)
       is deferred a few units into the next tile.
"""

import numpy as np

HEADS = 8
HD = 64
DIM = 512
DIN = 256
B, N, M, K = 4, 2048, 2048, 32
N_LOC = 1024  # queries per core
NT = N_LOC // 128  # query tiles per core
NCB = M // 128  # context blocks

_CACHE = {}


def _build():
    import concourse.bass as bass
    import concourse.bacc as bacc
    import concourse.mybir as mybir
    from concourse.tile import TileContext
    from concourse.masks import make_identity

    f32 = mybir.dt.float32
    bf16 = mybir.dt.bfloat16
    Act = mybir.ActivationFunctionType

    nc = bacc.Bacc("TRN2")
    xT_h = nc.dram_tensor("xT", [DIN, N_LOC], bf16, kind="ExternalInput")
    cT_h = nc.dram_tensor("cT", [DIN, M], bf16, kind="ExternalInput")
    cnt_h = nc.dram_tensor("cnt", [128, NT * NCB * 128], bf16, kind="ExternalInput")
    wqkv_h = nc.dram_tensor("wqkv", [DIN, 3 * DIM], bf16, kind="ExternalInput")
    wo_h = nc.dram_tensor("wo", [DIM, DIN], bf16, kind="ExternalInput")
    bq_h = nc.dram_tensor("bq", [64, 8], f32, kind="ExternalInput")
    bo_h = nc.dram_tensor("bo", [128, DIN], f32, kind="ExternalInput")
    out_h = nc.dram_tensor("out", [N_LOC, DIN], f32, kind="ExternalOutput")

    with TileContext(nc) as tc:
        with tc.tile_pool(name="const", bufs=1) as cpool:
            ident = cpool.tile([128, 128], bf16)
            make_identity(nc, ident[:])
            ones_sb = cpool.tile([128, 1], bf16)
            nc.vector.memset(ones_sb[:], 1.0)
            w_sb = [cpool.tile([128, 3 * DIM], bf16, tag=f"w{c}", name=f"w{c}") for c in range(2)]
            wo_sb = cpool.tile([128, 4 * DIN], bf16)
            bqc_sb = cpool.tile([64, 8], f32)
            bo_sb = cpool.tile([128, DIN], f32)
            for c in range(2):
                nc.sync.dma_start(out=w_sb[c][:], in_=wqkv_h[c * 128:(c + 1) * 128, :])
            # wo [512, 256] -> [128, (chunk, 256)] in one 3D-AP DMA
            nc.sync.dma_start(
                out=wo_sb[:].rearrange("p (c j) -> p c j", c=4),
                in_=wo_h[:].rearrange("(c p) j -> p c j", c=4))
            nc.sync.dma_start(out=bqc_sb[:], in_=bq_h[:])
            nc.sync.dma_start(out=bo_sb[:], in_=bo_h[:])

            def wq(c, h):
                return w_sb[c][:, h * 64:(h + 1) * 64]

            def wk(c, h):
                return w_sb[c][:, DIM + h * 64:DIM + (h + 1) * 64]

            def wv(c):
                return w_sb[c][:, 2 * DIM:3 * DIM]

            with tc.tile_pool(name="perm", bufs=1) as ppool:
                # per-head [64, .] tiles: matmul operands must start at
                # partition 0 (offset-64 operands fail on device)
                qT_sb = [ppool.tile([64, N_LOC], bf16, tag=f"qT{h}", name=f"qT{h}") for h in range(8)]
                kT_sb = [ppool.tile([64, M], bf16, tag=f"kT{h}", name=f"kT{h}") for h in range(8)]
                v_sb = [ppool.tile([128, DIM], bf16, tag=f"v{cb}", name=f"v{cb}") for cb in range(NCB)]

                # ---- phase A: projections ----
                with (
                    tc.tile_pool(name="inp", bufs=1) as ipool,
                    tc.tile_pool(name="psA", bufs=4, space="PSUM") as psA,
                ):
                    xT_sb = [ipool.tile([128, N_LOC], bf16, tag=f"xT{c}", name=f"xT{c}") for c in range(2)]
                    cT_sb = [ipool.tile([128, M], bf16, tag=f"cT{c}", name=f"cT{c}") for c in range(2)]
                    for c in range(2):
                        nc.scalar.dma_start(out=xT_sb[c][:], in_=xT_h[c * 128:(c + 1) * 128, :])
                        nc.scalar.dma_start(out=cT_sb[c][:], in_=cT_h[c * 128:(c + 1) * 128, :])
                    # kT_h[d, c] = Wk_h^T ctx^T (copies split DVE/Act)
                    for h in range(8):
                        for cc in range(4):
                            psk = psA.tile([64, 512], f32, tag="pAh", name="psk")
                            for c in range(2):
                                nc.tensor.matmul(
                                    out=psk[:], lhsT=wk(c, h),
                                    rhs=cT_sb[c][:, cc * 512:(cc + 1) * 512],
                                    start=(c == 0), stop=(c == 1))
                            dst = kT_sb[h][:, cc * 512:(cc + 1) * 512]
                            if cc % 2 == 0:
                                nc.vector.tensor_copy(out=dst, in_=psk[:])
                            else:
                                nc.scalar.activation(out=dst, in_=psk[:], func=Act.Copy)
                    # qT_h[d, q] = Wq_h^T x^T + bq (Act Identity folds bias)
                    for h in range(8):
                        for cc in range(2):
                            psq = psA.tile([64, 512], f32, tag="pAh", name="psq")
                            for c in range(2):
                                nc.tensor.matmul(
                                    out=psq[:], lhsT=wq(c, h),
                                    rhs=xT_sb[c][:, cc * 512:(cc + 1) * 512],
                                    start=(c == 0), stop=(c == 1))
                            nc.scalar.activation(
                                out=qT_sb[h][:, cc * 512:(cc + 1) * 512],
                                in_=psq[:], func=Act.Identity,
                                bias=bqc_sb[:, h:h + 1])
                    # V[c, hd] = ctx Wv
                    for cb in range(NCB):
                        psv = psA.tile([128, 512], f32, tag="pA", name="psv")
                        for c in range(2):
                            nc.tensor.matmul(
                                out=psv[:],
                                lhsT=cT_sb[c][:, cb * 128:(cb + 1) * 128],
                                rhs=wv(c),
                                start=(c == 0), stop=(c == 1))
                        nc.vector.tensor_copy(out=v_sb[cb][:], in_=psv[:])

                # ---- phase B: dense attention ----
                with (
                    tc.tile_pool(name="cntp", bufs=2) as cntp,
                    tc.tile_pool(name="atp", bufs=10) as atp,
                    tc.tile_pool(name="tailp", bufs=2) as tailp,
                    tc.tile_pool(name="psS", bufs=2, space="PSUM") as psS,
                    tc.tile_pool(name="psav", bufs=1, space="PSUM") as psav,
                    tc.tile_pool(name="psden", bufs=1, space="PSUM") as psden,
                    tc.tile_pool(name="psT", bufs=1, space="PSUM") as psT,
                    tc.tile_pool(name="psO", bufs=1, space="PSUM") as psO,
                ):
                    pend = []  # deferred tail2 closures

                    def tail2(t, avf, rden):
                        ao = tailp.tile([128, DIM], bf16, tag="ao", name=f"ao{t}")
                        with nc.allow_low_precision(reason="bf16 attention out"):
                            # inner-dim (d) stride-0 broadcast: gpsimd (DVE
                            # lacks it on hw; gpsimd can't read PSUM)
                            nc.gpsimd.tensor_tensor(
                                out=ao[:].rearrange("p (h d) -> p h d", h=8),
                                in0=avf[:].rearrange("p (h d) -> p h d", h=8),
                                in1=rden[:].rearrange("p (h o) -> p h o", o=1
                                                      ).to_broadcast([128, 8, 64]),
                                op=mybir.AluOpType.mult)
                        pst = psT.tile([128, DIM], bf16, tag="tp", name=f"pst{t}")
                        for c in range(4):
                            nc.tensor.transpose(
                                out=pst[:, c * 128:(c + 1) * 128],
                                in_=ao[:, c * 128:(c + 1) * 128],
                                identity=ident[:])
                        aT2 = tailp.tile([128, DIM], bf16, tag="aT2", name=f"aT2{t}")
                        nc.vector.tensor_copy(out=aT2[:], in_=pst[:])
                        op_ps = psO.tile([128, DIN], f32, tag="op", name=f"op{t}")
                        for c in range(4):
                            nc.tensor.matmul(
                                out=op_ps[:], lhsT=aT2[:, c * 128:(c + 1) * 128],
                                rhs=wo_sb[:, c * DIN:(c + 1) * DIN],
                                start=(c == 0), stop=(c == 3))
                        ot = tailp.tile([128, DIN], f32, tag="ot", name=f"ot{t}")
                        nc.vector.tensor_tensor(
                            out=ot[:], in0=op_ps[:], in1=bo_sb[:],
                            op=mybir.AluOpType.add)
                        nc.sync.dma_start(
                            out=out_h[t * 128:(t + 1) * 128, :], in_=ot[:])

                    for t in range(NT):
                        cnt_sb = cntp.tile([128, NCB * 128], bf16, tag="cnt", name=f"cnt{t}")
                        nc.sync.dma_start(
                            out=cnt_sb[:], in_=cnt_h[:, t * 2048:(t + 1) * 2048])
                        av_ps = psav.tile([128, DIM], f32, tag="av", name=f"av{t}")
                        den_ps = psden.tile([128, 8], f32, tag="den", name=f"den{t}")
                        for cb in range(NCB):
                            # flush the previous tile's deferred tail work
                            if cb == 2 and pend:
                                pend.pop()()
                            # S^T[c, (h,q)] for this (t, cb)
                            s_ps = psS.tile([128, 1024], f32, tag="s", name=f"s{t}_{cb}")
                            for h in range(8):
                                nc.tensor.matmul(
                                    out=s_ps[:, h * 128:(h + 1) * 128],
                                    lhsT=kT_sb[h][:, cb * 128:(cb + 1) * 128],
                                    rhs=qT_sb[h][:, t * 128:(t + 1) * 128],
                                    start=True, stop=True)
                            # A = exp(S^T) (Act), then A *= cnt (DVE)
                            slab = atp.tile([128, 8, 128], bf16, tag="at", name=f"at{t}_{cb}")
                            nc.scalar.activation(
                                out=slab[:].rearrange("p h q -> p (h q)"),
                                in_=s_ps[:], func=Act.Exp)
                            with nc.allow_low_precision(reason="bf16 attention weights"):
                                nc.vector.tensor_tensor(
                                    out=slab[:], in0=slab[:],
                                    in1=cnt_sb[:, cb * 128:(cb + 1) * 128].rearrange(
                                        "p (o q) -> p o q", o=1
                                    ).to_broadcast([128, 8, 128]),
                                    op=mybir.AluOpType.mult)
                            # av[q, (h,d)] += A_h^T V_h ; den[q, h] += A_h^T 1
                            # one PSUM accumulation group per bank: start on
                            # the first matmul, stop on the last.
                            for h in range(8):
                                nc.tensor.matmul(
                                    out=av_ps[:, h * 64:(h + 1) * 64],
                                    lhsT=slab[:, h, :],
                                    rhs=v_sb[cb][:, h * 64:(h + 1) * 64],
                                    start=(cb == 0 and h == 0),
                                    stop=(cb == NCB - 1 and h == 7))
                                nc.tensor.matmul(
                                    out=den_ps[:, h:h + 1],
                                    lhsT=slab[:, h, :],
                                    rhs=ones_sb[:],
                                    start=(cb == 0 and h == 0),
                                    stop=(cb == NCB - 1 and h == 7))
                        # tail1: free av/den PSUM fast (DVE only)
                        rden = tailp.tile([128, 8], f32, tag="rden", name=f"rden{t}")
                        nc.vector.reciprocal(out=rden[:], in_=den_ps[:])
                        avf = tailp.tile([128, DIM], f32, tag="avf", name=f"avf{t}")
                        nc.vector.tensor_copy(out=avf[:], in_=av_ps[:])
                        pend.append(lambda t=t, avf=avf, rden=rden: tail2(t, avf, rden))
                    pend.pop()()
    nc.compile()
    return nc


def _get_nc():
    if "nc" not in _CACHE:
        _CACHE["nc"] = _build()
    return _CACHE["nc"]


def kernel(**inputs) -> np.ndarray:
    from concourse.bass_utils import run_bass_kernel_spmd
    from ml_dtypes import bfloat16

    x = np.asarray(inputs["x"], dtype=np.float32)
    ctx = np.asarray(inputs["context"], dtype=np.float32)
    idx = np.asarray(inputs["index_pairs"]).astype(np.int64)
    scale = 1.0 / np.sqrt(HD)
    wq = np.asarray(inputs["Wq"], dtype=np.float32) * scale
    wqkv = np.concatenate(
        [wq, np.asarray(inputs["Wk"], dtype=np.float32),
         np.asarray(inputs["Wv"], dtype=np.float32)], axis=1).astype(bfloat16)
    bq = (np.asarray(inputs["bq"], dtype=np.float32) * scale).reshape(8, 64).T
    bq = np.ascontiguousarray(bq).astype(np.float32)  # [64, 8], col h = bq[h*64:(h+1)*64]
    wo = np.asarray(inputs["Wout"], dtype=np.float32).astype(bfloat16)
    bo = np.tile(np.asarray(inputs["bout"], dtype=np.float32).reshape(1, DIN),
                 (128, 1)).astype(np.float32)

    nc = _get_nc()
    in_maps = []
    qrep = np.repeat(np.arange(N_LOC), K)
    for c in range(8):
        b, half = c // 2, c % 2
        xT_c = np.ascontiguousarray(
            x[b, half * N_LOC:(half + 1) * N_LOC, :].T).astype(bfloat16)
        cT_c = np.ascontiguousarray(ctx[b].T).astype(bfloat16)
        idx_c = idx[b, half * N_LOC:(half + 1) * N_LOC, :]  # [1024, 32]
        # neighbor count matrix cnt[c, q], including duplicate multiplicity
        flat = idx_c.reshape(-1) * N_LOC + qrep
        cnt = np.bincount(flat, minlength=M * N_LOC).reshape(M, N_LOC)
        # layout [128 c-part, (t, cb, q)]
        cnt_w = np.ascontiguousarray(
            cnt.reshape(NCB, 128, NT, 128).transpose(1, 2, 0, 3).reshape(128, NT * NCB * 128)
        ).astype(bfloat16)
        in_maps.append({
            "xT": xT_c, "cT": cT_c, "cnt": cnt_w,
            "wqkv": wqkv, "wo": wo, "bq": bq, "bo": bo,
        })
    res = run_bass_kernel_spmd(nc, in_maps, core_ids=list(range(8)))
    out = np.empty((B, N, DIN), dtype=np.float32)
    for c in range(8):
        b, half = c // 2, c % 2
        out[b, half * N_LOC:(half + 1) * N_LOC, :] = res.results[c]["out"]
    return out
